# revision 8
# baseline (speedup 1.0000x reference)
"""Distance-weighted self-attention on 8 Trainium2 NeuronCores.

The reference network is rank-1 in the d_model dimension:
  q = h*Wq, k = h*Wk, v = h*Wv  (h = heights column of the input)
so  logits[s,t] = c*h_s*h_t - 0.5*|sz_s - sz_t|   with c = (Wq.Wk)/sqrt(256)
and out[s,:]   = (sum_t softmax(logits)[s,t]*h_t) * Wv.

Each core handles one batch element (B=8). Per core, for each 128-row block
of keys t (partitions) against all 2048 queries s (free dim):
  T2 = ch_rep * h_t[p]            (vector, fp16 4x)
  AD = |sh_rep - sh_t[p]|         (vector tensor_scalar chain: sub, abs_max 0)
  L  = T2 - AD                    (vector tensor_tensor, fp16 2x)
  E  = exp(L)                     (scalar engine)
  num/den via PE matmuls: lhsT=E[:,s-chunk], rhs=[h_t | 1]  -> PSUM [128, 32]
Then a = num/den and out chunks = a[p] * Wv_rep (outer product), DMA out.

Max-subtraction in softmax is unnecessary: |logits| <= ~12 and the common
factor cancels exactly in num/den.
"""

import os
import sys

import numpy as np

for _p in ("/opt/trn_rl_repo", "/root/.axon_site/_ro/trn_rl_repo"):
    if os.path.isdir(_p) and _p not in sys.path:
        sys.path.append(_p)

import concourse.bacc as bacc
import concourse.bass as bass
import concourse.mybir as mybir
import concourse.tile as tile
from concourse.bass_utils import run_bass_kernel_spmd
from concourse.dve_ops import (
    CUSTOM_DVE_SPECS,
    OPS,
    _CUSTOM_DVE_ROW_BASE,
    _SUB_OPCODE_FOR_NAME,
    DveOp,
)
from concourse.dve_spec import C0, C1, Spec, Src0, Src1, Zero, lower, maxx
from concourse.dve_table_gen import dve_ver_for
from concourse.dve_uop import DveOpSpec


def _register_logits_op() -> DveOp:
    """Fused DVE op: out[p,k] = in0[p,k]*s0[p] - |in1[p,k] - s1[p]|.

    One instruction per key-chunk computes the full logits block
    (rank-1 qk product minus the distance penalty), replacing a
    tensor_scalar + tensor_scalar + tensor_tensor chain.
    """
    name = "DWATT_LOGITS"
    existing = [op for op in OPS if op.name == name]
    if existing:
        return existing[0]
    d = Src1 - C1
    spec = Spec(
        body=Src0 * C0 - maxx(d, Zero - d),
        reference=lambda in0, in1, s0, s1, imm2: in0 * s0 - np.abs(in1 - s1),
    )
    opcode = _CUSTOM_DVE_ROW_BASE + len(OPS)
    assert opcode < 0x20
    shas = {}
    for ver in ("v3", "v4"):
        try:
            shas[ver] = DveOpSpec(
                name=name, opcode=opcode, uops=lower(spec, ver=ver), rd1_en=True
            ).sha(ver)
        except Exception:
            pass
    op = DveOp(name, spec, subdim=False, uops_sha=shas)
    OPS.append(op)
    _SUB_OPCODE_FOR_NAME[name] = opcode
    CUSTOM_DVE_SPECS[name] = spec
    return op


DWATT_LOGITS = _register_logits_op()

S = 2048
D = 256
P = 128
NJ = S // P  # 16
N_CORES = 8

f32 = mybir.dt.float32
f16 = mybir.dt.float16
Alu = mybir.AluOpType
Act = mybir.ActivationFunctionType


def build_kernel(nc: bass.Bass):
    x = nc.dram_tensor("x", [S, 2], f32, kind="ExternalInput").ap()
    wq = nc.dram_tensor("wq", [1, D], f32, kind="ExternalInput").ap()
    wk = nc.dram_tensor("wk", [1, D], f32, kind="ExternalInput").ap()
    wv = nc.dram_tensor("wv", [1, D], f32, kind="ExternalInput").ap()
    out = nc.dram_tensor("out", [S, D], f32, kind="ExternalOutput").ap()

    with tile.TileContext(nc) as tc:
        from contextlib import ExitStack

        with ExitStack() as ctx:
            const_pool = ctx.enter_context(tc.tile_pool(name="const", bufs=1))
            rep_pool = ctx.enter_context(tc.tile_pool(name="rep", bufs=1))
            work = ctx.enter_context(tc.tile_pool(name="work", bufs=2))
            epool = ctx.enter_context(tc.tile_pool(name="epool", bufs=2))
            outp = ctx.enter_context(tc.tile_pool(name="outp", bufs=2))
            ppsum = ctx.enter_context(
                tc.tile_pool(name="ppsum", bufs=1, space=bass.MemorySpace.PSUM)
            )
            mpsum = ctx.enter_context(
                tc.tile_pool(name="mpsum", bufs=1, space=bass.MemorySpace.PSUM)
            )

            # ---- load inputs --------------------------------------------
            # col_t[p, j, c]: x[128*j + p, c]  (c=0 sizes, c=1 heights)
            col_t = const_pool.tile([P, NJ, 2], f32)
            nc.sync.dma_start(col_t[:], x.rearrange("(j p) c -> p j c", p=P))
            colf = col_t[:].rearrange("p j c -> p (j c)")  # [128, 32]

            row_t = const_pool.tile([1, 2 * S], f32)
            nc.sync.dma_start(row_t[:], x.rearrange("s c -> (s c)").unsqueeze(0))
            sig_row = row_t[:, 0 : 2 * S : 2]  # [1, 2048]
            h_row = row_t[:, 1 : 2 * S : 2]  # [1, 2048]

            wq_t = const_pool.tile([1, D], f32)
            nc.sync.dma_start(wq_t[:], wq)
            wk_t = const_pool.tile([1, D], f32)
            nc.sync.dma_start(wk_t[:], wk)
            wv_t = const_pool.tile([1, D], f32)
            nc.sync.dma_start(wv_t[:], wv)

            # ---- c = (Wq . Wk) / 16 -------------------------------------
            wqk = const_pool.tile([1, D], f32)
            nc.vector.tensor_mul(wqk[:], wq_t[:], wk_t[:])
            c_red = const_pool.tile([1, 1], f32)
            nc.vector.tensor_reduce(c_red[:], wqk[:], axis=mybir.AxisListType.X, op=Alu.add)
            c_sb = const_pool.tile([1, 1], f32)
            nc.vector.tensor_scalar_mul(c_sb[:], c_red[:], 1.0 / 16.0)

            ones_row = const_pool.tile([1, P], f32)
            nc.vector.memset(ones_row[:], 1.0)
            c_col = const_pool.tile([1, P], f32)
            nc.vector.tensor_scalar_mul(c_col[:], ones_row[:], c_sb[:, 0:1])

            # ---- replicate rows across partitions via K=1 matmuls -------
            # ch_rep[p, s] = c*h[s], sh_rep[p, s] = 0.5*sizes[s]  (fp16)
            ps_rep = ppsum.tile([P, S], f32, tag="rep")
            for k in range(S // 512):
                nc.tensor.matmul(
                    ps_rep[:, 512 * k : 512 * (k + 1)],
                    c_col[:],
                    h_row[:, 512 * k : 512 * (k + 1)],
                    start=True,
                    stop=True,
                )
            ch_rep = rep_pool.tile([P, S], f32)
            nc.scalar.copy(ch_rep[:], ps_rep[:])

            ps_rep2 = ppsum.tile([P, S], f32, tag="rep")
            for k in range(S // 512):
                nc.tensor.matmul(
                    ps_rep2[:, 512 * k : 512 * (k + 1)],
                    ones_row[:],
                    sig_row[:, 512 * k : 512 * (k + 1)],
                    start=True,
                    stop=True,
                )
            sh_rep = rep_pool.tile([P, S], f32)
            nc.scalar.mul(sh_rep[:], ps_rep2[:], 0.5)

            # Wv replicated to all partitions (fp32, exact copy of Wv row)
            ps_w = ppsum.tile([P, D], f32, tag="wv")
            nc.tensor.matmul(ps_w[:], ones_row[:], wv_t[:], start=True, stop=True)
            wv_rep = const_pool.tile([P, D], f32)
            nc.vector.tensor_copy(wv_rep[:], ps_w[:])

            # ---- per-key-chunk scalars ----------------------------------
            # hones: cols 0..15 = h (fp16), cols 16..31 = 1.0
            hones = const_pool.tile([P, 2 * NJ], f16)
            nc.vector.tensor_copy(hones[:, 0:NJ], colf[:, 1 : 2 * NJ : 2])
            nc.vector.memset(hones[:, NJ : 2 * NJ], 1.0)
            sh_col = const_pool.tile([P, NJ], f32)
            nc.vector.tensor_scalar_mul(sh_col[:], colf[:, 0 : 2 * NJ : 2], 0.5)

            # ---- main loop over key chunks ------------------------------
            psum_nd = mpsum.tile([P, 2 * NJ], f32)
            nc.vector.memset(psum_nd[:], 0.0)
            for jt in range(NJ):
                lg = work.tile([P, S], f16, tag="lg")
                nc.vector._custom_dve(
                    DWATT_LOGITS,
                    out=lg[:],
                    in0=ch_rep[:],
                    in1=sh_rep[:],
                    s0=colf[:, 2 * jt + 1 : 2 * jt + 2],
                    s1=sh_col[:, jt : jt + 1],
                )
                ee = epool.tile([P, S], f16, tag="ee")
                nc.scalar.activation(ee[:], lg[:], Act.Exp)
                for js in range(NJ):
                    nc.tensor.matmul(
                        psum_nd[:, 2 * js : 2 * js + 2],
                        ee[:, P * js : P * (js + 1)],
                        hones[:, jt : jt + NJ + 1 : NJ],
                        start=False,
                        stop=(jt == NJ - 1),
                        skip_group_check=True,
                    )

            # ---- a = num/den, out = a * Wv ------------------------------
            nd_sb = const_pool.tile([P, 2 * NJ], f32)
            nc.scalar.copy(nd_sb[:], psum_nd[:])
            inv = const_pool.tile([P, NJ], f32)
            nc.vector.reciprocal(inv[:], nd_sb[:, 1 : 2 * NJ : 2])
            a_t = const_pool.tile([P, NJ], f32)
            nc.vector.tensor_mul(a_t[:], nd_sb[:, 0 : 2 * NJ : 2], inv[:])
            for j in range(NJ):
                ob = outp.tile([P, D], f32, tag="ob")
                nc.vector.tensor_scalar_mul(ob[:], wv_rep[:], a_t[:, j : j + 1])
                nc.sync.dma_start(out[P * j : P * (j + 1), :], ob[:])

    return nc


_NC = None


def _get_nc():
    global _NC
    if _NC is None:
        nc = bacc.Bacc("TRN2", target_bir_lowering=False, debug=False, num_devices=N_CORES)
        build_kernel(nc)
        nc.compile()
        _NC = nc
    return _NC


def kernel(inputs: np.ndarray, Wq: np.ndarray, Wk: np.ndarray, Wv: np.ndarray) -> np.ndarray:
    assert inputs.shape == (N_CORES, S, 2), inputs.shape
    nc = _get_nc()
    wq = np.ascontiguousarray(Wq, dtype=np.float32)
    wk = np.ascontiguousarray(Wk, dtype=np.float32)
    wv = np.ascontiguousarray(Wv, dtype=np.float32)
    in_maps = [
        {
            "x": np.ascontiguousarray(inputs[b], dtype=np.float32),
            "wq": wq,
            "wk": wk,
            "wv": wv,
        }
        for b in range(N_CORES)
    ]
    res = run_bass_kernel_spmd(nc, in_maps, core_ids=list(range(N_CORES)))
    return np.stack([r["out"] for r in res.results], axis=0)


# revision 33
# speedup vs baseline: 1.7333x; 1.7333x over previous
"""Distance-weighted self-attention on 8 Trainium2 NeuronCores.

The reference network is rank-1 in the d_model dimension:
  q = h*Wq, k = h*Wk, v = h*Wv  (h = heights column of the input)
so  logits[s,t] = c*h_s*h_t - 0.5*|sz_s - sz_t|   with c = (Wq.Wk)/sqrt(256)
and out[s,:]   = (sum_t softmax(logits)[s,t]*h_t) * Wv.

Each core handles one batch element (B=8). Per core, for each 128-row block
of keys t (partitions) against all 2048 queries s (free dim):
  L  = h_s_rep * (c*h_t[p]) - 0.5*|sig_s_rep - sig_t[p]|   (one fused DVE op)
  E  = exp(L)                                              (scalar engine)
  num/den via PE: lhsT=[h_t|1] stationary, rhs=E in 512-wide slices,
  accumulated over key chunks into PSUM rows [2, 2048].
Then num/den are transposed on-chip to [128, 32] (16 small PE matmuls against
a 2x2 identity accumulating into a zeroed PSUM bank), a = num/den, and
out chunks = a[p] * Wv_rep (outer products split across DVE and ACT), with
the 2MB result DMAed out in two halves on two queues.

Max-subtraction in softmax is unnecessary: |logits| <= ~12 and the common
factor cancels exactly in num/den.
"""

import os
import sys

import numpy as np

for _p in ("/opt/trn_rl_repo", "/root/.axon_site/_ro/trn_rl_repo"):
    if os.path.isdir(_p) and _p not in sys.path:
        sys.path.append(_p)

import concourse.bacc as bacc
import concourse.bass as bass
import concourse.mybir as mybir
import concourse.tile as tile
from concourse.bass_utils import run_bass_kernel_spmd
from concourse.dve_ops import (
    CUSTOM_DVE_SPECS,
    OPS,
    _CUSTOM_DVE_ROW_BASE,
    _SUB_OPCODE_FOR_NAME,
    DveOp,
)
from concourse.dve_spec import C0, C1, C2, Spec, Src0, Src1, Zero, lower, maxx
from concourse.dve_uop import DveOpSpec

S = 2048
D = 256
P = 128
NJ = S // P  # 16
N_CORES = 8

f32 = mybir.dt.float32
f16 = mybir.dt.float16
Alu = mybir.AluOpType
Act = mybir.ActivationFunctionType


def _register_logits_op() -> DveOp:
    """Fused DVE op: out[p,k] = in0[p,k]*s0[p] - |in1[p,k] - s1[p]|*imm2.

    One instruction per key-chunk computes the full logits block
    (rank-1 qk product minus the scaled distance penalty).
    """
    name = "DWATT_LOGITS"
    existing = [op for op in OPS if op.name == name]
    if existing:
        return existing[0]
    d = Src1 - C1
    spec = Spec(
        body=Src0 * C0 - maxx(d, Zero - d) * C2,
        reference=lambda in0, in1, s0, s1, imm2: in0 * s0 - np.abs(in1 - s1) * imm2,
    )
    opcode = _CUSTOM_DVE_ROW_BASE + len(OPS)
    assert opcode < 0x20
    shas = {}
    for ver in ("v3", "v4"):
        try:
            shas[ver] = DveOpSpec(
                name=name, opcode=opcode, uops=lower(spec, ver=ver), rd1_en=True
            ).sha(ver)
        except Exception:
            pass
    op = DveOp(name, spec, subdim=False, uops_sha=shas)
    OPS.append(op)
    _SUB_OPCODE_FOR_NAME[name] = opcode
    CUSTOM_DVE_SPECS[name] = spec
    return op


DWATT_LOGITS = _register_logits_op()


def build_kernel(nc: bass.Bass, repeat: int = 1):
    # x is the per-batch input TRANSPOSED on host: [2, S], row 0 = sizes,
    # row 1 = heights (contiguous rows enable broadcast/column DMAs).
    x = nc.dram_tensor("x", [2, S], f16, kind="ExternalInput").ap()
    wq = nc.dram_tensor("wq", [1, D], f32, kind="ExternalInput").ap()
    wk = nc.dram_tensor("wk", [1, D], f32, kind="ExternalInput").ap()
    wv = nc.dram_tensor("wv", [1, D], f32, kind="ExternalInput").ap()
    out = nc.dram_tensor("out", [S, D], f32, kind="ExternalOutput").ap()

    with tile.TileContext(nc) as tc:
        from contextlib import ExitStack

        with ExitStack() as ctx:
            const_pool = ctx.enter_context(tc.tile_pool(name="const", bufs=1))
            work = ctx.enter_context(tc.tile_pool(name="work", bufs=2))
            epool = ctx.enter_context(tc.tile_pool(name="epool", bufs=2))
            qpool = ctx.enter_context(tc.tile_pool(name="qpool", bufs=4))
            mpsum = ctx.enter_context(
                tc.tile_pool(name="mpsum", bufs=1, space=bass.MemorySpace.PSUM)
            )
            cpsum = ctx.enter_context(
                tc.tile_pool(name="cpsum", bufs=1, space=bass.MemorySpace.PSUM)
            )
            for _rep in range(repeat):
                _kernel_body(nc, tc, const_pool, work, epool, qpool, mpsum, cpsum, x, wq, wk, wv, out)

    return nc


def _kernel_body(nc, tc, const_pool, work, epool, qpool, mpsum, cpsum, x, wq, wk, wv, out):
    if True:
        if True:
            # ---- load inputs: weights first on the HWDGE queues, then the
            # two 1MB row broadcasts split in quarters across 4 queues ----
            wq_t = const_pool.tile([1, D], f32)
            nc.sync.dma_start(wq_t[:], wq)
            wk_t = const_pool.tile([1, D], f32)
            nc.scalar.dma_start(wk_t[:], wk)

            col3 = const_pool.tile([P, 2, NJ], f16)
            nc.gpsimd.dma_start(col3[:], x.rearrange("c (j p) -> p c j", p=P))
            colh = col3[:].rearrange("p c j -> p (c j)")  # [:, :16]=sig, [:, 16:]=h
            # f32 copy: per-partition scalar operands must be float32
            colft = const_pool.tile([P, 2 * NJ], f32)
            nc.vector.tensor_copy(colft[:], colh)
            colf = colft[:]
            wv_rep = const_pool.tile([P, D], f32)
            nc.gpsimd.dma_start(wv_rep[:], wv.to_broadcast([P, D]))

            # Replicated rows (every partition holds the full row).
            Q = S // 4
            sig_rep = const_pool.tile([P, S], f16)
            h_rep = const_pool.tile([P, S], f16)
            qeng = [nc.sync, nc.scalar, nc.sync, nc.scalar]
            for q in range(4):
                lo, hi = Q * q, Q * (q + 1)
                qeng[q].dma_start(sig_rep[:, lo:hi], x[0:1, lo:hi].to_broadcast([P, Q]))
                qeng[q + 1 if q % 2 == 0 else q - 1].dma_start(
                    h_rep[:, lo:hi], x[1:2, lo:hi].to_broadcast([P, Q])
                )

            # ---- c = (Wq . Wk) / 16, broadcast to all partitions --------
            wqk = const_pool.tile([1, D], f32)
            nc.vector.tensor_mul(wqk[:], wq_t[:], wk_t[:])
            c_red = const_pool.tile([1, 1], f32)
            nc.vector.tensor_reduce(c_red[:], wqk[:], axis=mybir.AxisListType.X, op=Alu.add)
            c_sb = const_pool.tile([1, 1], f32)
            nc.vector.tensor_scalar_mul(c_sb[:], c_red[:], 1.0 / 16.0)
            ones_row = const_pool.tile([1, P], f32)
            nc.vector.memset(ones_row[:], 1.0)
            psum_c = cpsum.tile([P, 1], f32, tag="c")
            nc.tensor.matmul(psum_c[:], ones_row[:], c_sb[:], start=True, stop=True)
            c_col = const_pool.tile([P, 1], f32)
            nc.vector.tensor_copy(c_col[:], psum_c[:])
            # ch_col[p, j] = c * h[128*j + p]
            ch_col = const_pool.tile([P, NJ], f32)
            nc.vector.tensor_scalar_mul(ch_col[:], colf[:, NJ : 2 * NJ], c_col[:])

            # hones: cols 0..15 = h chunks (fp16), cols 16..31 = 1.0
            hones = const_pool.tile([P, 2 * NJ], f16)
            nc.vector.tensor_copy(hones[:, 0:NJ], colh[:, NJ : 2 * NJ])
            nc.vector.memset(hones[:, NJ : 2 * NJ], 1.0)

            # 2x2 identity (stationary for the num/den transpose matmuls)
            i2 = const_pool.tile([2, 2], f32)
            nc.gpsimd.memset(i2[:], 1.0)
            nc.gpsimd.affine_select(
                out=i2[:],
                in_=i2[:],
                compare_op=Alu.is_equal,
                fill=0.0,
                base=0,
                pattern=[[-1, 2]],
                channel_multiplier=1,
            )

            # ---- main loop over key chunks ------------------------------
            # psum rows: 0 = num[s] (sum_t h_t*E), 1 = den[s] (sum_t E).
            # Each 512-col slice is exactly one PSUM bank, so per-slice
            # start=(jt==0) resets only its own bank.
            psum_nd = mpsum.tile([2, S], f32)
            nd_sb = const_pool.tile([2, S], f32)
            psum_t = cpsum.tile([P, 2 * NJ], f32, tag="t")
            nc.vector.memset(psum_t[:], 0.0)

            for jt in range(NJ - 1):
                lg = work.tile([P, S], f16, tag="lg")
                nc.vector._custom_dve(
                    DWATT_LOGITS,
                    out=lg[:],
                    in0=h_rep[:],
                    in1=sig_rep[:],
                    s0=ch_col[:, jt : jt + 1],
                    s1=colf[:, jt : jt + 1],
                    imm2=0.5,
                )
                ee = epool.tile([P, S], f16, tag="ee")
                nc.scalar.activation(ee[:], lg[:], Act.Exp)
                for ks in range(S // 512):
                    nc.tensor.matmul(
                        psum_nd[:, 512 * ks : 512 * (ks + 1)],
                        hones[:, jt : jt + NJ + 1 : NJ],
                        ee[:, 512 * ks : 512 * (ks + 1)],
                        start=(jt == 0),
                        stop=False,
                        skip_group_check=True,
                    )

            # Last key chunk runs in four 512-wide query quarters so the
            # num/den transpose + division can start per-quarter; the
            # transpose is 4 small PE matmuls against I2 accumulating into
            # a zeroed PSUM bank ([2,128] slice -> [128,2] columns).
            jt = NJ - 1
            for q in range(4):
                lo, hi = 512 * q, 512 * (q + 1)
                lgq = qpool.tile([P, 512], f16, tag="lgq")
                nc.vector._custom_dve(
                    DWATT_LOGITS,
                    out=lgq[:],
                    in0=h_rep[:, lo:hi],
                    in1=sig_rep[:, lo:hi],
                    s0=ch_col[:, jt : jt + 1],
                    s1=colf[:, jt : jt + 1],
                    imm2=0.5,
                )
                eeq = qpool.tile([P, 512], f16, tag="eeq")
                nc.scalar.activation(eeq[:], lgq[:], Act.Exp)
                nc.tensor.matmul(
                    psum_nd[:, lo:hi],
                    hones[:, jt : jt + NJ + 1 : NJ],
                    eeq[:],
                    start=False,
                    stop=True,
                    skip_group_check=True,
                )
                nc.scalar.copy(nd_sb[:, lo:hi], psum_nd[:, lo:hi])
                for j in range(4 * q, 4 * q + 4):
                    nc.tensor.matmul(
                        psum_t[:, 2 * j : 2 * j + 2],
                        nd_sb[:, P * j : P * (j + 1)],
                        i2[:],
                        start=False,
                        stop=(j == 2 * NJ - 1),
                        skip_group_check=True,
                    )

            # ---- per-quarter: a = num/den, out chunks = a * Wv, DMA -----
            out_sb = const_pool.tile([P, NJ * D], f32)
            out_r = out.rearrange("(j p) d -> p j d", p=P)
            ob3 = out_sb[:].rearrange("p (j d) -> p j d", d=D)
            nd_t = const_pool.tile([P, 2 * NJ], f32)
            inv = const_pool.tile([P, NJ], f32)
            a_t = const_pool.tile([P, NJ], f32)
            for q in range(4):
                c8 = nd_t[:, 8 * q : 8 * q + 8]
                nc.scalar.copy(c8, psum_t[:, 8 * q : 8 * q + 8])
                nc.vector.reciprocal(inv[:, 4 * q : 4 * q + 4], c8[:, 1:8:2])
                nc.vector.tensor_mul(
                    a_t[:, 4 * q : 4 * q + 4], c8[:, 0:8:2], inv[:, 4 * q : 4 * q + 4]
                )
                for j in range(4 * q, 4 * q + 4):
                    dst = out_sb[:, D * j : D * (j + 1)]
                    if j % 4 == 3 or j == 14:
                        nc.scalar.mul(dst, wv_rep[:], a_t[:, j : j + 1])
                    else:
                        nc.vector.tensor_scalar_mul(dst, wv_rep[:], a_t[:, j : j + 1])
                qeng[q].dma_start(
                    out_r[:, 4 * q : 4 * (q + 1)], ob3[:, 4 * q : 4 * (q + 1)]
                )


_NC = {}


def _get_nc(repeat: int = 1):
    if repeat not in _NC:
        nc = bacc.Bacc("TRN2", target_bir_lowering=False, debug=False, num_devices=N_CORES)
        build_kernel(nc, repeat)
        nc.compile()
        _NC[repeat] = nc
    return _NC[repeat]


def kernel(inputs: np.ndarray, Wq: np.ndarray, Wk: np.ndarray, Wv: np.ndarray) -> np.ndarray:
    assert inputs.shape == (N_CORES, S, 2), inputs.shape
    nc = _get_nc()
    wq = np.ascontiguousarray(Wq, dtype=np.float32)
    wk = np.ascontiguousarray(Wk, dtype=np.float32)
    wv = np.ascontiguousarray(Wv, dtype=np.float32)
    in_maps = [
        {
            "x": np.ascontiguousarray(np.asarray(inputs[b], dtype=np.float32).T.astype(np.float16)),
            "wq": wq,
            "wk": wk,
            "wv": wv,
        }
        for b in range(N_CORES)
    ]
    res = run_bass_kernel_spmd(nc, in_maps, core_ids=list(range(N_CORES)))
    return np.stack([r["out"] for r in res.results], axis=0)
